# revision 42
# baseline (speedup 1.0000x reference)
"""BigBird block-sparse attention forward on 8 Trainium2 NeuronCores (Bass/Tile).

Sharding: data-parallel over batch (2) x head-parallel (12 heads -> 4 groups of 3).
Core c handles batch c//4, heads [3*(c%4), 3*(c%4)+3).
Each core computes a partial output X_attn @ Wff[head_slice]; the host sums the
4 partials per batch and adds bff.

Shapes (hardcoded per the problem spec):
  X [2, 4096, 768], H=12 heads, D=64, block=64, n=64 blocks, 3 random blocks/row.
  mask is all-ones in this problem, so all mask terms vanish.

Numerics: bf16 matmul inputs, fp32 PSUM accumulation, exp on ScalarE in fp32.
Softmax skips max-subtraction (scores ~ N(0,1); exp is safe) so denominators
come free from a ones-column appended to V.

Random blocks are data-dependent, so under SPMD they are fetched with
indirect DMAs from a per-head DRAM table whose rows hold a PAIR of
consecutive tokens [K(2p)|K(2p+1)|V(2p),1|V(2p+1),1] (520B). Gathers are
ROW-ALIGNED: one 96-row indirect DMA per middle query row (the row's 3
random blocks as 96 token-pairs), so each gathered tile feeds exactly one
row's score/PV matmuls with no fragment splitting.
"""
import sys
sys.path.insert(0, "/opt/trn_rl_repo")
import numpy as np
import ml_dtypes

import concourse.bass as bass
import concourse.mybir as mybir
import concourse.tile as tile
from concourse.bass_utils import run_bass_kernel_spmd
from concourse.masks import make_identity

BF16 = mybir.dt.bfloat16
F32 = mybir.dt.float32
P = 128
B, N, DIM = 2, 4096, 768
H, D = 12, 64
BLK = 64
NB = N // BLK          # 64 blocks
R = 3
HPC = 3                # heads per core
NCORES = 8
KCH = 7                # contraction chunks: 768 dims + bias row, padded to 7*128
DIMP = KCH * P         # 896
NMID = NB - 2          # 62 middle rows (blocks 1..62)
NPAIR = NMID // 2      # 31 row pairs
GP = 96                # gathered token-pairs per middle row (3 blocks * 32)
SCALE = 0.125          # 1/sqrt(D)
import os
_NO_RAND = os.environ.get("KBUILD_NO_RAND", "0") == "1"   # bisection: skip random path
_RING = int(os.environ.get("KBUILD_RING", "48"))          # gather ring slots


def _split_excess_waits(nc, maxw=1):
    """This container's walrus accepts at most 1 sync wait per instruction.
    Hoist excess waits onto nofuse NoOps on the same engine just before."""
    n = 0
    for f in nc.m.functions:
        for bb in f.blocks:
            new_list = []
            changed = False
            for ins in bb.instructions:
                si = ins.sync_info
                w = list(si.on_wait) if si and si.on_wait else []
                if len(w) > maxw:
                    changed = True
                    excess, keep = w[:-maxw], w[-maxw:]
                    for i in range(0, len(excess), maxw):
                        nop = mybir.InstNoOp(name=f"{ins.name}-ws-{n}", engine=ins.engine)
                        nop.bass_nofuse = True
                        nop.sync_info = mybir.SyncInfo(on_wait=excess[i:i + maxw], on_update=[])
                        new_list.append(nop)
                        n += 1
                    ins.sync_info = mybir.SyncInfo(on_wait=keep, on_update=list(si.on_update or []))
                new_list.append(ins)
            if changed:
                bb.instructions = new_list
    return n


def _build_nc():
    nc = bass.Bass()
    # ---- inputs (per-core contents differ, program is SPMD-uniform) ----
    xt = nc.declare_dram_parameter("xt", [DIMP, N], BF16, isOutput=False)        # X[b].T + ones row + zero pad
    wa = nc.declare_dram_parameter("wa", [DIMP, P], BF16, isOutput=False)        # [Wq h0 | Wq h1] (+bias row)
    wb = nc.declare_dram_parameter("wb", [DIMP, P], BF16, isOutput=False)        # [Wq h2 | Wk h2]
    wc = nc.declare_dram_parameter("wc", [DIMP, P], BF16, isOutput=False)        # [Wk h0 | Wk h1]
    wv = nc.declare_dram_parameter("wv", [DIMP, 3 * P], BF16, isOutput=False)    # [Wv h0..h2 | Wk h0..h2]
    wf1 = nc.declare_dram_parameter("wf1", [P, DIM], BF16, isOutput=False)       # Wff rows hd 0:128
    wf2 = nc.declare_dram_parameter("wf2", [D, DIM], BF16, isOutput=False)       # Wff rows hd 128:192
    gidx = nc.declare_dram_parameter("gidx", [GP, HPC, NMID], mybir.dt.int32, isOutput=False)
    out = nc.declare_dram_parameter("out", [N, DIM], BF16, isOutput=True)        # partial output

    # internal DRAM: per-head tables, pair rows [K(2p)|K(2p+1)|V(2p),1|V(2p+1),1|pad]
    tbls = [nc.dram_tensor(f"tbl{h}", [N // 2, 260], BF16) for h in range(HPC)]

    with tile.TileContext(nc) as tc:
      with tc.tile_pool(name="persist", bufs=1) as sb_persist:
        ident = sb_persist.tile([P, P], BF16)
        make_identity(nc, ident[:])

        # per-head persistent tiles; qt is duplicated [QT; QT] so the odd-token
        # half of a transposed gathered-K tile can be lhsT at base partition 64
        qt_h = [sb_persist.tile([P, N], BF16, name=f"qt{h}") for h in range(HPC)]
        kt_h = [sb_persist.tile([D, N], BF16, name=f"kt{h}") for h in range(HPC)]
        # V token-major per head: [tok-in-pair, pair, 65] (col 64 = ones)
        v_h = [sb_persist.tile([P, NB // 2, 65], BF16, name=f"v{h}") for h in range(HPC)]
        # normalized context, all heads, SWAPPED chunk layout:
        # chunk c partitions 0:64 = block 2c+1 tokens, 64:128 = block 2c tokens
        ctx = sb_persist.tile([P, NB // 2, HPC * D], BF16, name="ctx")
        vfl_h = [sb_persist.tile([P, 65], BF16, name=f"vfl{h}") for h in range(HPC)]
        idx_sb = sb_persist.tile([GP, HPC, NMID], mybir.dt.int32)
        nc.sync.dma_start(idx_sb[:], gidx[:])
        wf1s = sb_persist.tile([P, DIM], BF16)
        nc.sync.dma_start(wf1s[:], wf1[:])
        wf2s = sb_persist.tile([D, DIM], BF16)
        nc.sync.dma_start(wf2s[:], wf2[:])

        _ktm_cm = tc.tile_pool(name="ktmpool", bufs=1)
        sbk2 = _ktm_cm.__enter__()
        ktm_h = [sbk2.tile([P, NB // 2, D], BF16, name=f"ktm{h}") for h in range(HPC)]

        # ---------------- Phase B: projections ----------------
        with tc.tile_pool(name="proj", bufs=1) as sbp, \
             tc.tile_pool(name="projp", bufs=2, space="PSUM") as psp:
            xts = sbp.tile([P, KCH, N], BF16)
            nc.sync.dma_start(xts[:], xt[:].rearrange("(ko p) n -> p ko n", p=P))
            was = sbp.tile([P, KCH, P], BF16)
            nc.sync.dma_start(was[:], wa[:].rearrange("(ko p) m -> p ko m", p=P))
            wbs = sbp.tile([P, KCH, P], BF16)
            nc.sync.dma_start(wbs[:], wb[:].rearrange("(ko p) m -> p ko m", p=P))
            wcs = sbp.tile([P, KCH, P], BF16)
            nc.sync.dma_start(wcs[:], wc[:].rearrange("(ko p) m -> p ko m", p=P))
            wvs = sbp.tile([P, KCH, 3 * P], BF16)
            nc.sync.dma_start(wvs[:], wv[:].rearrange("(ko p) m -> p ko m", p=P))

            # V+K(tok-major) projection: out [tok 128, 384]; V tiles get ones col
            for h in range(HPC):
                nc.vector.memset(v_h[h][:], 1.0)
            for nt2 in range(NB // 2):
                acc = psp.tile([P, 3 * P], F32, tag="prjv")
                for ko in range(KCH):
                    nc.tensor.matmul(acc[:], xts[:, ko, nt2 * P:(nt2 + 1) * P],
                                     wvs[:, ko], start=(ko == 0), stop=(ko == KCH - 1))
                for h in range(HPC):
                    nc.vector.tensor_copy(v_h[h][:, nt2, 0:D], acc[:, h * D:(h + 1) * D])
                    nc.vector.tensor_copy(ktm_h[h][:, nt2], acc[:, 192 + h * D:192 + (h + 1) * D])
            # Q/K transposed projections: out [128 = two 64-d slots, Ntok]
            for (wt, dst0, dst1) in ((was, ("q", 0), ("q", 1)),
                                     (wbs, ("q", 2), ("k", 2)),
                                     (wcs, ("k", 0), ("k", 1))):
                for nt in range(8):
                    acc = psp.tile([P, 512], F32, tag="prj")
                    for ko in range(KCH):
                        nc.tensor.matmul(acc[:], wt[:, ko], xts[:, ko, nt * 512:(nt + 1) * 512],
                                         start=(ko == 0), stop=(ko == KCH - 1))
                    for slot, (kind, hh) in enumerate((dst0, dst1)):
                        sl = slice(nt * 512, (nt + 1) * 512)
                        if kind == "q":
                            nc.vector.tensor_copy(qt_h[hh][0:D, sl], acc[slot * D:(slot + 1) * D])
                            nc.vector.tensor_copy(qt_h[hh][D:P, sl], acc[slot * D:(slot + 1) * D])
                        else:
                            nc.vector.tensor_copy(kt_h[hh][:, sl], acc[slot * D:(slot + 1) * D])

        # pair-table writes + V_fl
        for h in range(HPC):
            # pair row r = blk*32 + i holds tokens (64*blk + i, 64*blk + i + 32);
            # r = 64t + 32s + i with blk = 2t + s, so all DMAs read contiguous
            # partition ranges of the token-major tiles.
            dst = tbls[h][:].rearrange("(t s i) e -> s i t e", s=2, i=32)
            nc.sync.dma_start(dst[0, :, :, 0:64], ktm_h[h][0:32])
            nc.sync.dma_start(dst[1, :, :, 0:64], ktm_h[h][64:96])
            nc.sync.dma_start(dst[0, :, :, 64:128], ktm_h[h][32:64])
            nc.sync.dma_start(dst[1, :, :, 64:128], ktm_h[h][96:128])
            nc.sync.dma_start(dst[0, :, :, 128:193], v_h[h][0:32])
            nc.sync.dma_start(dst[1, :, :, 128:193], v_h[h][64:96])
            nc.sync.dma_start(dst[0, :, :, 193:258], v_h[h][32:64])
            nc.sync.dma_start(dst[1, :, :, 193:258], v_h[h][96:128])
            # V_fl = [V block0 | V block63] rows with ones col
            nc.vector.tensor_copy(vfl_h[h][0:D], v_h[h][0:D, 0])
            nc.vector.tensor_copy(vfl_h[h][D:P], v_h[h][D:P, NB // 2 - 1])
        _ktm_cm.__exit__(None, None, None)

        # ---------------- gathers (issued early, drain in background) ----------------
        kv_sel = {}
        with tc.tile_pool(name="gath", bufs=_RING) as sbg:
            if not _NO_RAND:
                for h in range(HPC):
                    for l in range(1, NMID + 1):
                        t = sbg.tile([GP, 260], BF16, tag="kv")
                        nc.gpsimd.indirect_dma_start(
                            out=t[:], out_offset=None, in_=tbls[h][:],
                            in_offset=bass.IndirectOffsetOnAxis(ap=idx_sb[:, h, l - 1:l], axis=0))
                        kv_sel[(h, l)] = t

            # ---------------- Phase C: attention per head ----------------
            with tc.tile_pool(name="attn", bufs=2) as sba_big, \
                 tc.tile_pool(name="attnr", bufs=7) as sba, \
                 tc.tile_pool(name="attnp", bufs=3, space="PSUM") as psa, \
                 tc.tile_pool(name="accp", bufs=3, space="PSUM") as psacc, \
                 tc.tile_pool(name="trp", bufs=2, space="PSUM") as pstr:
                for h in range(HPC):
                    qt, kt = qt_h[h], kt_h[h]

                    # contiguous [d, 128] tiles holding q/k cols of blocks {0, 63}
                    qfl = sba.tile([D, P], BF16, tag="qfl")
                    nc.vector.tensor_copy(qfl[:, 0:BLK], qt[0:D, 0:BLK])
                    nc.vector.tensor_copy(qfl[:, BLK:P], qt[0:D, (NB - 1) * BLK:N])
                    kfl = sba.tile([D, P], BF16, tag="kfl")
                    nc.vector.tensor_copy(kfl[:, 0:BLK], kt[:, 0:BLK])
                    nc.vector.tensor_copy(kfl[:, BLK:P], kt[:, (NB - 1) * BLK:N])

                    # ---- full rows 0 & 63: ST [128 keys-chunk, 128 q(2 rows)] ----
                    expf = sba_big.tile([P, NB // 2, P], BF16, tag="expf")
                    for c2 in range(16):
                        stp = psa.tile([P, 2, P], F32, tag="st")
                        for j in range(2):
                            c = 2 * c2 + j
                            nc.tensor.matmul(stp[:, j], kt[:, c * P:(c + 1) * P], qfl[:],
                                             start=True, stop=True)
                        nc.scalar.activation(expf[:, 2 * c2:2 * c2 + 2, :], stp[:],
                                             mybir.ActivationFunctionType.Exp, scale=SCALE)
                    cfull = psacc.tile([P, 65], F32, tag="cacc")
                    for c in range(NB // 2):
                        nc.tensor.matmul(cfull[:], expf[:, c, :], v_h[h][:, c, 0:65],
                                         start=(c == 0), stop=(c == NB // 2 - 1))
                    rec = sba.tile([P, 1], F32, tag="recf")
                    nc.vector.reciprocal(rec[:], cfull[:, 64:65])
                    # swapped ctx layout: block 0 -> chunk0 partitions 64:128,
                    # block 63 -> chunk31 partitions 0:64  (cross-partition muls)
                    nc.vector.tensor_scalar_mul(ctx[D:P, 0, h * D:(h + 1) * D],
                                                cfull[0:D, 0:D], rec[0:D])
                    nc.vector.tensor_scalar_mul(ctx[0:D, NB // 2 - 1, h * D:(h + 1) * D],
                                                cfull[D:P, 0:D], rec[D:P])

                    # ---- global scores: keys = blocks {0, 63}, all q ----
                    expg = sba_big.tile([P, N], BF16, tag="expg")
                    for c in range(8):
                        stp = psa.tile([P, 512], F32, tag="st")
                        nc.tensor.matmul(stp[:], kfl[:], qt[0:D, c * 512:(c + 1) * 512],
                                         start=True, stop=True)
                        nc.scalar.activation(expg[:, c * 512:(c + 1) * 512], stp[:],
                                             mybir.ActivationFunctionType.Exp, scale=SCALE)

                    # ---- window score tiles (ring) ----
                    expw = [None] * (NB // 2)

                    def window_tile(g, qt=qt, kt=kt):
                        kb_lo = max(2 * g, 1)
                        kb_hi = min(2 * g + 1, NB - 2)
                        q_lo = max(2 * g - 1, 1)
                        q_hi = min(2 * g + 2, NB - 2)
                        nq = (q_hi - q_lo + 1) * BLK
                        st = psa.tile([P, 256], F32, tag="st")
                        part0 = (kb_lo % 2) * BLK
                        nk = (kb_hi - kb_lo + 1) * BLK
                        nc.tensor.matmul(st[part0:part0 + nk, 0:nq],
                                         kt[:, kb_lo * BLK:(kb_hi + 1) * BLK],
                                         qt[0:D, q_lo * BLK:(q_hi + 1) * BLK],
                                         start=True, stop=True)
                        ew = sba.tile([P, 256], BF16, tag="expw")
                        nc.scalar.activation(ew[part0:part0 + nk, 0:nq],
                                             st[part0:part0 + nk, 0:nq],
                                             mybir.ActivationFunctionType.Exp,
                                             scale=SCALE)
                        for xi in range(2):
                            x = 2 * g + xi
                            if x < kb_lo or x > kb_hi:
                                nc.vector.memset(ew[xi * BLK:(xi + 1) * BLK, :], 0.0)
                                continue
                            for li in range(4):
                                l = 2 * g - 1 + li
                                col = (l - q_lo) * BLK
                                if l < q_lo or l > q_hi:
                                    continue
                                if abs(l - x) > 1:
                                    nc.vector.memset(
                                        ew[xi * BLK:(xi + 1) * BLK, col:col + BLK], 0.0)
                        if nq < 256:
                            nc.vector.memset(ew[:, nq:256], 0.0)
                        return ew, q_lo

                    for g0 in range(4):
                        expw[g0] = window_tile(g0)

                    # ---- random row processing: transpose + scores + exp ----
                    er_t = {}

                    def process_pair_randoms(m, h=h, qt=qt):
                        if _NO_RAND:
                            return
                        """Transposed-K scores for rows ra=2m+1, rb=2m+2 into one
                        exp tile er [96, 4, 64]: cols 0,1 = ra (even,odd tokens),
                        cols 2,3 = rb."""
                        st = psa.tile([GP, 4, BLK], F32, tag="st")
                        for half, l in ((0, 2 * m + 1), (1, 2 * m + 2)):
                            kv = kv_sel[(h, l)]
                            # ONE transpose covers both 64-col K halves
                            # (out parts 0:64 = even-token K^T, 64:128 = odd);
                            # the PSUM->SBUF copies drop both halves to base 0
                            # so no lhsT sits at base 64 (that + quadrant PVs
                            # hangs the PE)
                            tp = pstr.tile([P, GP], BF16, tag="rtr")
                            nc.tensor.transpose(tp[:], kv[:, 0:P], ident[0:GP, 0:GP])
                            kts = sba.tile([D, 2, GP], BF16, tag="kts")
                            nc.vector.tensor_copy(kts[:, 0, :], tp[0:D, :])
                            nc.vector.tensor_copy(kts[:, 1, :], tp[D:P, :])
                            nc.tensor.matmul(st[:, 2 * half + 0, :], kts[:, 0, :],
                                             qt[0:D, l * BLK:(l + 1) * BLK],
                                             start=True, stop=True)
                            nc.tensor.matmul(st[:, 2 * half + 1, :], kts[:, 1, :],
                                             qt[0:D, l * BLK:(l + 1) * BLK],
                                             start=True, stop=True)
                        er = sba.tile([GP, 4, BLK], BF16, tag="expr")
                        nc.scalar.activation(er[:], st[:], mybir.ActivationFunctionType.Exp,
                                             scale=SCALE)
                        er_t[m] = er

                    process_pair_randoms(0)

                    for m in range(NPAIR):
                        if m + 4 < NB // 2:
                            expw[m + 4] = window_tile(m + 4)
                        if m + 1 < NPAIR:
                            process_pair_randoms(m + 1)
                        ra, rb = 2 * m + 1, 2 * m + 2

                        cacc = psacc.tile([P, 65], F32, tag="cacc")
                        nc.tensor.matmul(cacc[:], expg[:, ra * BLK:(ra + 2) * BLK], vfl_h[h][:],
                                         start=True, stop=False)
                        ew_m, qlo_m = expw[m]
                        col = (ra - qlo_m) * BLK
                        nc.tensor.matmul(cacc[:], ew_m[:, col:col + 2 * BLK], v_h[h][:, m, 0:65],
                                         start=False, stop=False)
                        ew_n, qlo_n = expw[m + 1]
                        col = (ra - qlo_n) * BLK
                        nc.tensor.matmul(cacc[:], ew_n[:, col:col + 2 * BLK],
                                         v_h[h][:, m + 1, 0:65], start=False, stop=False)
                        # random PV: per row, even-token & odd-token gathered V
                        if _NO_RAND:
                            nc.tensor.matmul(cacc[:], ew_m[:, col:col + 2 * BLK],
                                             v_h[h][:, m, 0:65], start=False, stop=True)
                        else:
                            er = er_t.pop(m)
                            for half, l in ((0, ra), (1, rb)):
                                kv = kv_sel[(h, l)]
                                nc.tensor.matmul(cacc[half * D:(half + 1) * D],
                                                 er[:, 2 * half + 0, :], kv[:, 128:193],
                                                 start=False, stop=False,
                                                 tile_position=(0, half * D))
                                nc.tensor.matmul(cacc[half * D:(half + 1) * D],
                                                 er[:, 2 * half + 1, :], kv[:, 193:258],
                                                 start=False, stop=True,
                                                 tile_position=(0, half * D))

                        recp = sba.tile([P, 1], F32, tag="recp")
                        nc.vector.reciprocal(recp[:], cacc[:, 64:65])
                        # swapped ctx layout: ra (block 2m+1) -> chunk m lo;
                        # rb (block 2m+2) -> chunk m+1 hi.  No partition shifts.
                        nc.vector.tensor_scalar_mul(ctx[0:D, m, h * D:(h + 1) * D],
                                                    cacc[0:D, 0:D], recp[0:D])
                        nc.vector.tensor_scalar_mul(ctx[D:P, m + 1, h * D:(h + 1) * D],
                                                    cacc[D:P, 0:D], recp[D:P])

        # ---------------- Phase D: ctx transpose + output projection ----------------
        with tc.tile_pool(name="outp", bufs=2) as sbo, \
             tc.tile_pool(name="outpp", bufs=2, space="PSUM") as pso, \
             tc.tile_pool(name="outtr", bufs=2, space="PSUM") as pstr2:
            for nt2 in range(NB // 2):
                tpd = pstr2.tile([P, 256], BF16, tag="ctr")
                nc.tensor.transpose(tpd[:, 0:128], ctx[:, nt2, 0:128], ident[:])
                nc.tensor.transpose(tpd[0:D, 128:256], ctx[:, nt2, 128:192], ident[:])
                ct01 = sbo.tile([P, P], BF16, tag="ct01")
                nc.vector.tensor_copy(ct01[:], tpd[:, 0:128])
                ct2 = sbo.tile([D, P], BF16, tag="ct2")
                nc.vector.tensor_copy(ct2[:], tpd[0:D, 128:256])
                acc1 = pso.tile([P, 512], F32, tag="o1")
                acc2 = pso.tile([P, 256], F32, tag="o2")
                nc.tensor.matmul(acc1[:], ct01[:], wf1s[:, 0:512], start=True, stop=False)
                nc.tensor.matmul(acc1[:], ct2[:], wf2s[:, 0:512], start=False, stop=True)
                nc.tensor.matmul(acc2[:], ct01[:], wf1s[:, 512:768], start=True, stop=False)
                nc.tensor.matmul(acc2[:], ct2[:], wf2s[:, 512:768], start=False, stop=True)
                ostage = sbo.tile([P, DIM], BF16, tag="ostage")
                nc.vector.tensor_copy(ostage[:, 0:512], acc1[:])
                nc.vector.tensor_copy(ostage[:, 512:768], acc2[:])
                # swapped chunk layout: partitions 0:64 = block 2*nt2+1 tokens
                nc.sync.dma_start(out[nt2 * P + D:nt2 * P + P, :], ostage[0:D])
                nc.sync.dma_start(out[nt2 * P:nt2 * P + D, :], ostage[D:P])
    return nc


_CACHE = {}


def _prep_core_inputs(X, rand_attn, Wq, bq, Wk, bk, Wv, bv, Wff):
    """Host-side sharding: returns list of 8 input dicts."""
    bf = ml_dtypes.bfloat16
    in_maps = []
    for core in range(NCORES):
        b = core // 4
        g = core % 4
        hs = [3 * g, 3 * g + 1, 3 * g + 2]

        xtm = np.zeros((DIMP, N), np.float32)
        xtm[:768] = X[b].T
        xtm[768] = 1.0  # bias row
        xtm = xtm.astype(bf)

        def wslice(W, bvec, h):
            col = np.zeros((DIMP, D), np.float32)
            col[:768] = W[:, h * D:(h + 1) * D]
            col[768] = bvec[h * D:(h + 1) * D]
            return col

        wa = np.concatenate([wslice(Wq, bq, hs[0]), wslice(Wq, bq, hs[1])], 1).astype(bf)
        wb = np.concatenate([wslice(Wq, bq, hs[2]), wslice(Wk, bk, hs[2])], 1).astype(bf)
        wc = np.concatenate([wslice(Wk, bk, hs[0]), wslice(Wk, bk, hs[1])], 1).astype(bf)
        wv_ = np.concatenate([wslice(Wv, bv, hs[0]), wslice(Wv, bv, hs[1]),
                              wslice(Wv, bv, hs[2]), wslice(Wk, bk, hs[0]),
                              wslice(Wk, bk, hs[1]), wslice(Wk, bk, hs[2])], 1).astype(bf)
        wff_rows = Wff[np.concatenate([np.arange(h * D, (h + 1) * D) for h in hs])]  # [192, 768]
        wf1 = wff_rows[0:P].astype(bf)
        wf2 = wff_rows[P:P + D].astype(bf)

        # gather indices: per middle row l, its 96 token-pair rows (3 blocks x 32)
        gidx = np.zeros((GP, HPC, NMID), np.int32)
        for hh, h in enumerate(hs):
            # pairs[l-1, r*32+i] = rand_attn[h, l-1, r]*32 + i
            pairs = (rand_attn[h][:, :, None] * (BLK // 2)
                     + np.arange(BLK // 2)[None, None, :]).reshape(NMID, GP)
            gidx[:, hh, :] = pairs.T
        in_maps.append(dict(xt=xtm, wa=wa, wb=wb, wc=wc, wv=wv_, wf1=wf1, wf2=wf2, gidx=gidx))
    return in_maps


def kernel(X, mask, rand_attn, Wq, bq, Wk, bk, Wv, bv, Wff, bff):
    X = np.asarray(X, np.float32)
    rand_attn = np.asarray(rand_attn, np.int32)
    in_maps = _prep_core_inputs(X, rand_attn, np.asarray(Wq, np.float32), np.asarray(bq, np.float32),
                                np.asarray(Wk, np.float32), np.asarray(bk, np.float32),
                                np.asarray(Wv, np.float32), np.asarray(bv, np.float32),
                                np.asarray(Wff, np.float32))
    if "nc" not in _CACHE:
        nc = _build_nc()
        _split_excess_waits(nc)
        _CACHE["nc"] = nc
    res = run_bass_kernel_spmd(_CACHE["nc"], in_maps, core_ids=list(range(NCORES)))
    out = np.zeros((B, N, DIM), np.float32)
    for core in range(NCORES):
        out[core // 4] += res.results[core]["out"].astype(np.float32)
    out += np.asarray(bff, np.float32)[None, None, :]
    return out

